# revision 50
# baseline (speedup 1.0000x reference)
"""Trainium2 Bass kernel for nn_Attention_62749472195138.

Dense transformer attention block:
  LayerNorm(C) -> 1x1 conv QKV -> l2norm(q,k over tokens) -> softmax(q k^T * 10) v
  -> 1x1 conv out + bias

Sharding: pure data-parallel over batch B=8 across the 8 NeuronCores (one
batch element per core, weights replicated, no collectives).

Per-core shapes: x [N=1024, C=512]; heads=8, dim_head=64.

The kernel is a single software pipeline governed by the ACT (scalar)
engine, which carries the irreducible exp() work of the softmax
(64 x [128,1024] tiles ~= 66us of ACT time).  Everything else hides under
the exp stream.  PSUM bank budget (8 x 2KB):

  sim pool   4 banks  2-deep ring of [128,1024]f32; consumer is ONLY the
                      ACT exp, so the exp stream never waits on DVE
  av pool    2 banks  [65,512]f32 attn@v accumulators for one head pair,
                      one i-half at a time (halves run back to back)
  work pool  2 banks  2-deep ring of [128,512]f32 for q/k/v projection
                      half-chains and out-projection chunks (DVE-drained)

Per slot p (16 exps of head-pair p) the PE interleaves: attn@v chains for
pair p (trailing the exps by one j-chunk), sim refills, q/k projection
half-chains for pair p+1, and the second-i-half attn@v chains of pair p-1.
DVE handles all PSUM drains (gpsimd cannot touch PSUM) plus LayerNorm and
l2norm stats; rsqrt/reciprocal run as Newton iterations / the custom
approx op so ACT stays exp-only (no activation-table reloads).

Other tricks: l2norm of q AND k folded into one per-partition scale on q
(rq*rk/N with the 1/N constant folded into the scale op), ones-column in
v_aug so attn@v also yields softmax denominators, denominator broadcast
via DMA round-trip through DRAM, out-projection bias via a K=1 matmul.
"""

import os
import numpy as np
import ml_dtypes

import concourse.bass as bass
import concourse.tile as tile
from concourse import mybir, bacc
from concourse.bass_utils import run_bass_kernel_spmd
from concourse.masks import make_identity

F32 = mybir.dt.float32
BF16 = mybir.dt.bfloat16
AF = mybir.ActivationFunctionType
ALU = mybir.AluOpType

N = 1024          # tokens per batch element (32*32)
C = 512           # channels
HEADS = 8
DH = 64           # dim per head
PAIRS = HEADS // 2
SCALE = 10.0
LN_EPS = 1e-5
NCHUNK = N // 128  # 8 token chunks
CCHUNK = C // 128  # 4 channel chunks
NCORES = 8


def build_graph():
    nc = bacc.Bacc()

    x_ext = nc.declare_dram_parameter("x", [N, C], BF16, isOutput=False)
    wqk_ext = nc.declare_dram_parameter("w_qk", [C, 2 * C], BF16, isOutput=False)
    wv_ext = nc.declare_dram_parameter("w_v", [C, C], BF16, isOutput=False)
    wo_ext = nc.declare_dram_parameter("w_out", [C, C], BF16, isOutput=False)
    bo_ext = nc.declare_dram_parameter("b_out", [1, C], BF16, isOutput=False)
    out_ext = nc.declare_dram_parameter("out", [N, C], F32, isOutput=True)

    with tile.TileContext(nc) as tc:
        with (
            tc.tile_pool(name="consts", bufs=1) as consts,
            tc.tile_pool(name="persist", bufs=1) as persist,
            tc.tile_pool(name="xin", bufs=1) as xin,
            tc.tile_pool(name="stats", bufs=4) as stats,
            tc.tile_pool(name="l2p", bufs=2) as l2p,
            tc.tile_pool(name="atp", bufs=20) as atp,
            tc.tile_pool(name="rbp", bufs=4) as rbp,
            tc.tile_pool(name="avsbp", bufs=4) as avsbp,
            tc.tile_pool(name="rdp", bufs=8, space="DRAM") as rdp,
            tc.tile_pool(name="t1p", bufs=2) as t1p,
            tc.tile_pool(name="ftp", bufs=3) as ftp,
            tc.tile_pool(name="sim_ps", bufs=2, space="PSUM") as sim_ps,
            tc.tile_pool(name="av_ps", bufs=2, space="PSUM") as av_ps,
            tc.tile_pool(name="work_ps", bufs=2, space="PSUM") as work_ps,
        ):
            dma_qs = [nc.sync, nc.scalar, nc.gpsimd, nc.sync]

            # ---- constants / inputs --------------------------------------
            ident = consts.tile([128, 128], BF16)
            make_identity(nc, ident)
            # Warm the PE clock gate (HAM) while DMAs/LN run: ~4us of junk
            # transposes take the PE from 1.2 to 2.4 GHz before real matmuls.
            warm = work_ps.tile([128, 128], BF16, tag="work", name="warm")
            for _ in range(56):
                nc.tensor.transpose(warm, ident, ident)
            x_ts = []
            for ic in range(NCHUNK):
                x_t = xin.tile([128, C], BF16, name=f"x{ic}", tag=f"x{ic}")
                dma_qs[ic % 3].dma_start(out=x_t, in_=x_ext[ic * 128:(ic + 1) * 128, :])
                x_ts.append(x_t)
            bias_row = consts.tile([1, C], BF16)
            nc.sync.dma_start(out=bias_row, in_=bo_ext[:, :])
            ones_row = consts.tile([1, 128], BF16)
            nc.vector.memset(ones_row, 1.0)

            w_qk = persist.tile([128, CCHUNK, 2 * C], BF16)   # [c%128, cc, f]
            w_v = persist.tile([128, CCHUNK, C], BF16)        # [c%128, cc, vf]
            w_o = persist.tile([128, CCHUNK, C], BF16)        # [f%128, fc, c]
            for cc in range(CCHUNK):
                nc.scalar.dma_start(out=w_qk[:, cc, :], in_=wqk_ext[cc * 128:(cc + 1) * 128, :])
                nc.gpsimd.dma_start(out=w_v[:, cc, :], in_=wv_ext[cc * 128:(cc + 1) * 128, :])
                nc.gpsimd.dma_start(out=w_o[:, cc, :], in_=wo_ext[cc * 128:(cc + 1) * 128, :])

            # persistent activations
            yT = persist.tile([128, CCHUNK, N], BF16)          # [c%128, cc, i]
            qkT = persist.tile([128, 2 * CCHUNK, N], BF16)     # [f%128, fc, i]; fc<4 q, fc>=4 k
            v_aug = persist.tile([128, NCHUNK, HEADS, DH + 1], BF16)  # [j%128, jc, h, d|1]
            outT = persist.tile([128, CCHUNK, N], BF16)        # [f%128, fc, i]
            nc.vector.memset(v_aug[:, :, :, DH:DH + 1], 1.0)

            def newton_rsqrt(pool, src, width, name, iters=3):
                """rsqrt via linear init + Newton iterations, pure DVE.

                Accurate to ~1e-5 for src in roughly [0.5, 2]; all tiles are
                [128, width] so everything runs as tiny DVE ops.  Keeps the
                ACT engine exp-only (a Ln/Rsqrt there forces ~1.3us table
                reloads mid-stream).
                """
                y = pool.tile([128, width], F32, tag=f"{name}y", name=f"{name}y")
                nc.vector.tensor_scalar(out=y, in0=src, scalar1=-0.5,
                                        scalar2=1.5, op0=ALU.mult, op1=ALU.add)
                for it in range(iters):
                    y2 = pool.tile([128, width], F32, tag=f"{name}2",
                                   name=f"{name}2_{it}")
                    nc.vector.tensor_tensor(out=y2, in0=y, in1=y, op=ALU.mult)
                    nc.vector.tensor_tensor(out=y2, in0=src, in1=y2, op=ALU.mult)
                    nc.vector.tensor_scalar(out=y2, in0=y2, scalar1=-0.5,
                                            scalar2=1.5, op0=ALU.mult, op1=ALU.add)
                    nc.vector.tensor_tensor(out=y, in0=y, in1=y2, op=ALU.mult)
                return y

            # ---- LayerNorm + transpose (ramp), two groups of 4 chunks ----
            def ln_group(g):
                ics = range(4 * g, 4 * g + 4)
                mv = stats.tile([128, 4, 2], F32, tag=f"mv{g}", bufs=1,
                                name=f"mv{g}")
                for i, ic in enumerate(ics):
                    st = stats.tile([128, 6], F32, tag=f"st{ic % 4}", name=f"st{ic}")
                    nc.vector.bn_stats(out=st, in_=x_ts[ic])
                    nc.vector.bn_aggr(out=mv[:, i, :], in_=st)
                vpe = stats.tile([128, 4], F32, tag=f"vpe{g}", bufs=1,
                                 name=f"vpe{g}")
                nc.vector.tensor_scalar(out=vpe, in0=mv[:, :, 1], scalar1=1.0,
                                        scalar2=LN_EPS, op0=ALU.mult, op1=ALU.add)
                rstd = newton_rsqrt(stats, vpe, 4, f"rstd{g}", iters=2)
                nmr = stats.tile([128, 4], F32, tag=f"nmr{g}", bufs=1,
                                 name=f"nmr{g}")
                nc.vector.tensor_tensor(out=nmr, in0=mv[:, :, 0], in1=rstd,
                                        op=ALU.mult)
                nc.vector.tensor_scalar_mul(out=nmr, in0=nmr, scalar1=-1.0)
                for i, ic in enumerate(ics):
                    y_t = stats.tile([128, C], BF16, tag="y", name=f"y{ic}")
                    nc.vector.tensor_scalar(out=y_t, in0=x_ts[ic],
                                            scalar1=rstd[:, i:i + 1],
                                            scalar2=nmr[:, i:i + 1],
                                            op0=ALU.mult, op1=ALU.add)
                    pt = sim_ps.tile([128, CCHUNK, 128], BF16, tag="sim",
                                     name=f"pt{ic}")
                    for cc in range(CCHUNK):
                        nc.tensor.transpose(pt[:, cc, :],
                                            y_t[:, cc * 128:(cc + 1) * 128], ident)
                    nc.vector.tensor_copy(out=yT[:, :, ic * 128:(ic + 1) * 128],
                                          in_=pt)

            # ---- pipeline helpers ----------------------------------------
            sim_tiles = {}   # (jc, s) -> psum tile (current pair only)
            at_tiles = {}    # (jc, s) -> sbuf bf16 tile (current pair)
            av_tiles = {}    # s -> psum accumulator [65, 512] (current half)

            def proj_qk_half(fc, half):
                """project a 128-row chunk of q or k for one i-half"""
                hs = slice(half * 512, (half + 1) * 512)
                pq = work_ps.tile([128, C], F32, tag="work", name=f"pq{fc}_{half}")
                for cc in range(CCHUNK):
                    nc.tensor.matmul(
                        pq,
                        lhsT=w_qk[:, cc, fc * 128:(fc + 1) * 128],
                        rhs=yT[:, cc, hs],
                        start=(cc == 0), stop=(cc == CCHUNK - 1),
                    )
                nc.vector.tensor_copy(out=qkT[:, fc, hs], in_=pq)

            def vproj(jc):
                pv = work_ps.tile([128, C], F32, tag="work", name=f"pv{jc}")
                for cc in range(CCHUNK):
                    nc.tensor.matmul(
                        pv,
                        lhsT=yT[:, cc, jc * 128:(jc + 1) * 128],
                        rhs=w_v[:, cc, :],
                        start=(cc == 0), stop=(cc == CCHUNK - 1),
                    )
                nc.vector.tensor_copy(
                    out=v_aug[:, jc, :, 0:DH],
                    in_=pv.rearrange("p (h d) -> p h d", h=HEADS),
                )

            l2_sts = {}

            def l2_stats(hp, idx, half):
                """bn_stats of one i-half of a q (idx=0) / k (idx=1) row chunk"""
                if (hp, idx) not in l2_sts:
                    l2_sts[(hp, idx)] = l2p.tile([128, 2, 6], F32,
                                                 tag=f"lst{idx}",
                                                 name=f"lst{hp}_{idx}")
                fc = hp + CCHUNK * idx
                nc.vector.bn_stats(out=l2_sts[(hp, idx)][:, half, :],
                                   in_=qkT[:, fc, half * 512:(half + 1) * 512])

            def l2_fold(hp):
                """fold rq*rk/N into q in-place (ssq = N*(var + mean^2))."""
                mv = l2p.tile([128, 2, 2], F32, tag="mv", name=f"mv{hp}")
                for idx in range(2):
                    nc.vector.bn_aggr(out=mv[:, idx, :],
                                      in_=l2_sts.pop((hp, idx)))
                sq = l2p.tile([128, 2], F32, tag="ssq", name=f"ssq{hp}")
                nc.vector.tensor_tensor(out=sq, in0=mv[:, :, 0], in1=mv[:, :, 0],
                                        op=ALU.mult)
                nc.vector.tensor_tensor(out=sq, in0=sq, in1=mv[:, :, 1],
                                        op=ALU.add)
                ry = newton_rsqrt(l2p, sq, 2, f"nq{hp}", iters=2)
                rqk = l2p.tile([128, 1], F32, tag="rqk", name=f"rqk{hp}")
                nc.vector.tensor_tensor(out=rqk, in0=ry[:, 0:1], in1=ry[:, 1:2],
                                        op=ALU.mult)
                # the in-place scale gates the next pair's sims: run it on
                # gpsimd (empty queue) so it can't sit behind DVE work
                nc.gpsimd.tensor_scalar(out=qkT[:, hp, :], in0=qkT[:, hp, :],
                                        scalar1=rqk, scalar2=1.0 / N,
                                        op0=ALU.mult, op1=ALU.mult)

            def sim_mm(hp, jc):
                """row-packed sim matmuls for (pair hp, j-chunk jc)"""
                for s in range(2):
                    t = sim_ps.tile([128, N], F32, tag="sim",
                                    name=f"sim{hp}_{jc}_{s}")
                    psl = slice(s * 64, (s + 1) * 64)
                    for half in range(2):
                        hs = slice(half * 512, (half + 1) * 512)
                        nc.tensor.matmul(
                            t[:, hs],
                            lhsT=qkT[psl, CCHUNK + hp, jc * 128:(jc + 1) * 128],
                            rhs=qkT[psl, hp, hs],
                            start=True, stop=True,
                        )
                    sim_tiles[(jc, s)] = t

            def exp_mm(hp, jc):
                for s in range(2):
                    at = atp.tile([128, N], BF16, tag="at", name=f"at{hp}_{jc}_{s}")
                    nc.scalar.activation(out=at, in_=sim_tiles.pop((jc, s)),
                                         func=AF.Exp, scale=SCALE)
                    at_tiles[(hp, jc, s)] = at

            avb_tiles = {}   # pair-3 second-half accumulators (work pool)

            def av_alloc(half):
                for s in range(2):
                    av_tiles[s] = av_ps.tile([DH + 1, 512], F32, tag="av",
                                             name=f"av{s}_{half}")

            def avb_alloc():
                # slot 3 has no next-pair projections, so its work-pool
                # banks can hold the second-half accumulators and let both
                # halves of pair 3 finish with the last exp
                for s in range(2):
                    avb_tiles[s] = work_ps.tile([DH + 1, 512], F32, tag="work",
                                                name=f"avb{s}")

            def av_mm(hp, jc, half, pop, tiles=None):
                if tiles is None:
                    tiles = av_tiles
                hs = slice(half * 512, (half + 1) * 512)
                for s in range(2):
                    at = at_tiles.pop((hp, jc, s)) if pop else at_tiles[(hp, jc, s)]
                    nc.tensor.matmul(
                        tiles[s][:, :],
                        lhsT=v_aug[:, jc, 2 * hp + s, :],
                        rhs=at[:, hs],
                        start=(jc == 0), stop=(jc == NCHUNK - 1),
                    )

            def normalize(hp, half, tiles=None):
                """softmax denominators -> outT for (pair hp, i-half)"""
                if tiles is None:
                    tiles = av_tiles
                hs = slice(half * 512, (half + 1) * 512)
                for s in range(2):
                    # drain av psum to SBUF with ONE fast DVE copy so the
                    # psum ring recycles immediately; the denominator
                    # broadcast/reciprocal/normalize then run off the
                    # critical path from the SBUF copy.
                    av = tiles.pop(s)
                    av_sb = avsbp.tile([DH + 1, 512], F32, tag="avsb",
                                       name=f"avsb{hp}_{half}_{s}")
                    nc.vector.tensor_copy(out=av_sb, in_=av)
                    # separate DMA queues per s so one pair's bounce can't
                    # head-of-line block the other's
                    q = nc.sync if s == 0 else nc.gpsimd
                    rd_d = rdp.tile([1, 512], F32, tag="rd",
                                    name=f"rdd{hp}_{half}_{s}")
                    q.dma_start(out=rd_d, in_=av_sb[DH:DH + 1, :])
                    rb_raw = rbp.tile([DH, 512], F32, tag="rbr",
                                      name=f"rbr{hp}_{half}_{s}")
                    rd_b = bass.AP(tensor=rd_d.tensor, offset=rd_d.offset,
                                   ap=[[0, DH]] + rd_d.ap[1:])
                    q.dma_start(out=rb_raw, in_=rd_b)
                    rb = rbp.tile([DH, 512], F32, tag="rb",
                                  name=f"rb{hp}_{half}_{s}")
                    nc.vector.reciprocal_approx_fast(out=rb, in_=rb_raw)
                    # all operands SBUF now -> the multiply can run on the
                    # otherwise-idle gpsimd engine
                    if s == 0:
                        nc.gpsimd.tensor_tensor(out=outT[0:DH, hp, hs],
                                                in0=av_sb[0:DH, :], in1=rb,
                                                op=ALU.mult)
                    else:
                        t1 = t1p.tile([DH, 512], BF16, tag="t1",
                                      name=f"t1{hp}_{half}")
                        nc.gpsimd.tensor_tensor(out=t1, in0=av_sb[0:DH, :],
                                                in1=rb, op=ALU.mult)
                        q.dma_start(out=outT[DH:128, hp, hs], in_=t1)

            def oproj_chunk(ic):
                """full out-projection chain for one token chunk + bias"""
                po = work_ps.tile([128, C], F32, tag="work", name=f"po{ic}")
                nc.tensor.matmul(po, lhsT=ones_row, rhs=bias_row,
                                 start=True, stop=False)
                for fc in range(CCHUNK):
                    nc.tensor.matmul(
                        po,
                        lhsT=outT[:, fc, ic * 128:(ic + 1) * 128],
                        rhs=w_o[:, fc, :],
                        start=False, stop=(fc == CCHUNK - 1),
                    )
                f_t = ftp.tile([128, C], F32, tag="fin", name=f"fin{ic}")
                # ACT is idle at the tail; Copy shares the exp table
                nc.scalar.copy(out=f_t, in_=po)
                eng = nc.sync if ic % 2 == 0 else nc.gpsimd
                eng.dma_start(out=out_ext[ic * 128:(ic + 1) * 128, :], in_=f_t)

            # ---- ramp: LN group 0 -> first projections while group 1 runs
            ln_group(0)
            proj_qk_half(0, 0)
            proj_qk_half(CCHUNK, 0)
            ln_group(1)
            l2_stats(0, 0, 0)
            l2_stats(0, 1, 0)
            proj_qk_half(0, 1)
            proj_qk_half(CCHUNK, 1)
            l2_stats(0, 0, 1)
            l2_stats(0, 1, 1)
            vproj(0)
            vproj(1)
            l2_fold(0)
            av_alloc(0)
            sim_mm(0, 0)
            sim_mm(0, 1)

            # ---- main pipeline: 4 head-pair slots ------------------------
            # Slot p: exps of pair p; attn@v first-half chains trail by one
            # j-chunk; second-half chains of pair p-1 run early in slot p;
            # q/k projections for pair p+1 and v projections (slot 0) fill
            # the PE between sims.
            for p in range(PAIRS):
                for jc in range(NCHUNK):
                    exp_mm(p, jc)
                    if jc <= 5:
                        sim_mm(p, jc + 2)
                    # second i-half of previous pair, 4 j-chunks per step
                    if p > 0 and jc in (0, 1):
                        for jc2 in range(4 * jc, 4 * jc + 4):
                            av_mm(p - 1, jc2, 1, pop=True)
                        if jc == 1:
                            normalize(p - 1, 1)
                            av_alloc(0)
                    if jc >= 1:
                        av_mm(p, jc - 1, 0, pop=False)
                        if p == PAIRS - 1:
                            if jc == 1:
                                avb_alloc()
                            av_mm(p, jc - 1, 1, pop=True, tiles=avb_tiles)
                    if p == 0 and jc <= 5:
                        vproj(jc + 2)
                    if p < PAIRS - 1:
                        # q/k projection + l2 stats for pair p+1, spread so
                        # the in-place q scale lands well before the next
                        # slot's first sims
                        if jc == 1:
                            proj_qk_half(p + 1, 0)
                        elif jc == 2:
                            proj_qk_half(p + 1, 1)
                            l2_stats(p + 1, 0, 0)
                        elif jc == 3:
                            proj_qk_half(CCHUNK + p + 1, 0)
                            l2_stats(p + 1, 0, 1)
                        elif jc == 4:
                            proj_qk_half(CCHUNK + p + 1, 1)
                            l2_stats(p + 1, 1, 0)
                        elif jc == 5:
                            l2_stats(p + 1, 1, 1)
                            l2_fold(p + 1)
                        elif jc == 6:
                            sim_mm(p + 1, 0)
                        elif jc == 7:
                            sim_mm(p + 1, 1)
                # end of slot: finish first-half chain, start second half
                av_mm(p, NCHUNK - 1, 0, pop=False)
                if p == PAIRS - 1:
                    av_mm(p, NCHUNK - 1, 1, pop=True, tiles=avb_tiles)
                normalize(p, 0)
                if p < PAIRS - 1:
                    av_alloc(1)

            # ---- tail: normalize pair-3 second half, out-projection ------
            normalize(PAIRS - 1, 1, tiles=avb_tiles)
            # chunks 0-3 read only outT[...,0:512] (ready after the
            # first-half normalizes); 4-7 wait on the line above
            for ic in range(NCHUNK):
                oproj_chunk(ic)

    nc.finalize()
    return nc


_GRAPH = None


def kernel(x, ln_scale, w_qkv, w_out, b_out):
    global _GRAPH
    B, H, W, Cc = x.shape
    assert (B, H * W, Cc) == (NCORES, N, C)

    # fold LayerNorm scale into the QKV weight (diag(ln_scale) @ w_qkv)
    w = ln_scale.astype(np.float32)[:, None] * np.asarray(w_qkv, np.float32)
    bf = ml_dtypes.bfloat16
    w_qk_h = np.ascontiguousarray(w[:, : 2 * C]).astype(bf)
    w_v_h = np.ascontiguousarray(w[:, 2 * C:]).astype(bf)
    w_o_h = np.asarray(w_out, np.float32).astype(bf)
    b_o_h = np.asarray(b_out, np.float32).reshape(1, C).astype(bf)

    if _GRAPH is None:
        _GRAPH = build_graph()

    in_maps = [
        {
            "x": np.ascontiguousarray(x[b].reshape(N, C)).astype(bf),
            "w_qk": w_qk_h,
            "w_v": w_v_h,
            "w_out": w_o_h,
            "b_out": b_o_h,
        }
        for b in range(B)
    ]
    trace = bool(int(os.environ.get("BASS_KERNEL_TRACE", "0")))
    kw = {}
    if trace:
        kw["trace"] = True
        td = os.environ.get("BASS_KERNEL_TRACE_DIR")
        if td:
            kw["tmpdir"] = td
    res = run_bass_kernel_spmd(_GRAPH, in_maps, core_ids=list(range(NCORES)), **kw)
    if trace:
        print(f"HW exec time: {res.exec_time_ns} ns")
    out = np.stack([res.results[b]["out"].reshape(H, W, C) for b in range(B)])
    return out.astype(np.float32)
